# revision 28
# baseline (speedup 1.0000x reference)
"""Expert-parallel MoE MLP + residual + LayerNorm on 8 Trainium2 NeuronCores.

Reference computes a dense all-expert MLP then masks: out[t] only depends on
expert e = mask[t].  We route: core d gets expert d's weights plus the tokens
assigned to expert d (gathered on host, zero-padded to a fixed capacity C),
computes gelu(x@w1+b1)@w2+b2, adds the residual, applies LayerNorm, and the
host scatters rows back.  No collectives needed: each token's output lives on
exactly one core.

Per-core design:
  matmul1 in fp8e4m3 DoubleRow (2 contraction rows per partition, 256-deep
  per instruction): interT[i, t] = sum_h w1[h, i] * x[t, h].  w1 is scaled
  x32 on the host so its N(0, 0.02) weights clear the e4m3 subnormal range;
  the gelu ACT un-scales via its input-scale argument (gelu(ps/32 + b1)).
  The short tail block (free dim < 128) instead uses 6 plain 128-deep fp8
  matmuls over the same packed data — DoubleRow's doubled LDWEIGHTS makes it
  a net loss at small free dims.
  matmul2 in bf16, token-major: y[t, h] = sum_i interT[i, t] * w2[i, h],
  PSUM split (512, 256) along H (one-bank accumulation-group limit), with
  the k-loop OUTER so the two PSUM regions' matmuls run back-to-back on the
  same lhsT (the interT chunk) — measured ~8us faster on HW than region-
  outer ordering (the second weight load is avoided or hidden).
  LayerNorm: residual-add fused with the row-sum (scalar_tensor_tensor
  accum_out), Square+row-sum on ACT (same act table as gelu), stats batched
  per 512-token block on [P, n_chunks] tiles, rsqrt(var+eps) on the DVE via
  the bit-trick + 2 Newton steps (keeps Sqrt's table reload off the ACT
  stream), and the final normalize as an ACT Identity with per-row
  scale/bias pointers.  When ln_gamma==1 and ln_beta==0 the post-LN affine
  is skipped entirely (fast path; general inputs take the 2-TT path).
  Each finished block's normalize is emitted after the NEXT block's
  mm1/gelu batch so it fills the matmul2 window instead of stalling the
  in-order ACT stream; the final block's mm1 is hoisted before the previous
  block's last matmul2 chunk for the same reason.
  DMA: w1 streams on SP HWDGE in i-chunks sized to the PE's consumption
  (first chunk small for a fast start); xgt/b1 descriptor-gen rides the
  otherwise-idle GpSimd SWDGE in parallel; w2 follows w1 (needed when
  matmul2 starts); xres (bf16, b2 pre-added) and gamma/beta arrive last.
"""

import numpy as np
import ml_dtypes

import concourse.bacc as bacc
import concourse.mybir as mybir
import concourse.tile as tile
from concourse.bass_utils import run_bass_kernel_spmd

E, T, H, I = 8, 8192, 768, 3072
P = 128
HK2 = H // 256  # 3 double-row contraction groups for matmul1
IK = I // P  # 24
W1SPLIT = (512, 1280, 1280)  # w1 DMA i-chunk widths (first small: fast start)
EPS = 1e-12
W1S = 32.0  # host-side w1 scale (un-done by the gelu ACT input scale)
N_CORES = 8

F32 = mybir.dt.float32
BF16 = mybir.dt.bfloat16
FP8 = mybir.dt.float8e4
I32 = mybir.dt.int32
AF = mybir.ActivationFunctionType
ALU = mybir.AluOpType
DR = mybir.MatmulPerfMode.DoubleRow

MAGIC = 0x5F3759DF
MM2_K_OUTER = True  # k-outer keeps consecutive matmuls on the same lhsT


def _build(C: int, act=AF.Gelu, reps: int = 1, n_tok: int | None = None, affine: bool = True):
    """C: DRAM capacity (multiple of 128). n_tok: tokens actually computed
    (n_tok <= C); the tail beyond n_tok is padding nobody reads back."""
    if n_tok is None:
        n_tok = C
    TCN = C // P
    blocks = []
    off = 0
    while off < n_tok:
        tb = min(512, n_tok - off)
        blocks.append((off, tb))
        off += tb

    nc = bacc.Bacc(None, target_bir_lowering=False)

    xgt_d = nc.dram_tensor("xgt", [HK2, P, 2, C], FP8, kind="ExternalInput")
    xres_d = nc.dram_tensor("xres", [TCN, P, H], BF16, kind="ExternalInput")
    w1_d = nc.dram_tensor("w1", [HK2, P, 2, I], FP8, kind="ExternalInput")
    b1t_d = nc.dram_tensor("b1t", [P, IK], F32, kind="ExternalInput")
    w2_d = nc.dram_tensor("w2", [IK, P, H], BF16, kind="ExternalInput")
    gb_d = nc.dram_tensor("gb", [P, 2, H], F32, kind="ExternalInput") if affine else None
    out_d = nc.dram_tensor("out", [TCN, P, H], F32, kind="ExternalOutput")

    with tile.TileContext(nc) as tc:
        with (
            tc.tile_pool(name="res", bufs=1) as rpool,
            tc.tile_pool(name="ln", bufs=2) as lnpool,
            tc.tile_pool(name="small", bufs=2) as spool,
            tc.tile_pool(name="psA", bufs=4, space="PSUM") as ppa,
            tc.tile_pool(name="psB", bufs=2, space="PSUM") as ppb,
        ):
            # constants for the DVE rsqrt (bit-trick + Newton)
            magici = rpool.tile([P, 4], I32)
            nc.gpsimd.memset(magici[:], MAGIC)
            onesi = rpool.tile([P, 4], I32)
            nc.gpsimd.memset(onesi[:], 1)
            c15 = rpool.tile([P, 4], F32)
            nc.gpsimd.memset(c15[:], 1.5)

            b1sb = rpool.tile([P, IK], F32)
            gbsb = rpool.tile([P, 2, H], F32) if affine else None

            for _rep in range(reps):
                w1sb = [
                    rpool.tile([P, 2, I], FP8, tag=f"w1_{k}", name=f"w1sb{k}")
                    for k in range(HK2)
                ]
                xgtsb = [
                    rpool.tile([P, 2, C], FP8, tag=f"xgt_{k}", name=f"xgtsb{k}")
                    for k in range(HK2)
                ]
                w2sb = [
                    rpool.tile([P, H], BF16, tag=f"w2_{k}", name=f"w2sb{k}")
                    for k in range(IK)
                ]
                xressb = [
                    rpool.tile([P, H], BF16, tag=f"xres_{c}", name=f"xressb{c}")
                    for c in range(TCN)
                ]
                interT = rpool.tile([P, IK, C], BF16, tag="interT", name="interT")

                # critical-path DMAs first: w1 chunk0 + bias + xgt unblock the
                # first psum group ~5us in; later w1 chunks stream just ahead
                # of the PE's m-group consumption; w2 must land by the time
                # matmul2 starts (~14us); xres/gb only matter for LayerNorm.
                # xgt + b1 ride the Pool SWDGE: descriptor generation runs on
                # the otherwise-idle GpSimd engine, in parallel with the
                # HWDGE generation of the w1 stream on SP.
                for k in range(HK2):
                    nc.gpsimd.dma_start(xgtsb[k][:], xgt_d[k])
                nc.gpsimd.dma_start(b1sb[:], b1t_d[:])
                ioff = 0
                for w in W1SPLIT:
                    for k in range(HK2):
                        nc.sync.dma_start(
                            w1sb[k][:, :, ioff : ioff + w],
                            w1_d[k][:, :, ioff : ioff + w],
                        )
                    ioff += w
                for k2 in range(IK):
                    nc.sync.dma_start(w2sb[k2][:], w2_d[k2])
                for c in range(TCN):
                    nc.sync.dma_start(xressb[c][:], xres_d[c])
                if affine:
                    nc.sync.dma_start(gbsb[:], gb_d[:])

                def emit_normalize(boff, tb, nch, xs, yt, nmr):
                    """Per-chunk normalize + store for a finished block.
                    Emitted AFTER the next block's mm1/gelu batch so the ACT
                    affine ops never stall the gelu stream."""
                    for tci in range(nch):
                        tcg = boff // P + tci
                        toff = tci * P
                        tw = min(P, tb - toff)
                        # normalize-affine on the ACT engine:
                        # o = Identity(x * rs + nmr) with per-row scale/bias ptrs
                        o = lnpool.tile([P, H], F32, tag="o")
                        nc.scalar.activation(
                            o[:tw],
                            xs[tci][:tw],
                            AF.Identity,
                            bias=nmr[:tw, tci : tci + 1],
                            scale=yt[:tw, tci : tci + 1],
                        )
                        if affine:
                            nc.vector.tensor_mul(o[:tw], o[:tw], gbsb[:tw, 0, :])
                            nc.vector.tensor_add(o[:tw], o[:tw], gbsb[:tw, 1, :])
                        nc.sync.dma_start(out_d[tcg][:tw], o[:tw])

                def emit_mm1(boff, tb):
                    # DoubleRow only beats plain fp8 (which runs at bf16 speed
                    # with automatic FWL) when the moving free dim is >=128;
                    # for the short tail block fall back to 6 plain 128-deep
                    # accumulating matmuls over the same DoubleRow-packed data.
                    use_dr = tb >= 128
                    for m in range(IK):
                        ps = ppa.tile([P, 512], F32, tag="psA")
                        if use_dr:
                            for k in range(HK2):
                                nc.tensor.matmul(
                                    ps[:, :tb],
                                    w1sb[k][:, :, m * P : (m + 1) * P],
                                    xgtsb[k][:, :, boff : boff + tb],
                                    start=(k == 0),
                                    stop=(k == HK2 - 1),
                                    perf_mode=DR,
                                )
                        else:
                            for k in range(HK2):
                                for j in range(2):
                                    nc.tensor.matmul(
                                        ps[:, :tb],
                                        w1sb[k][:, j, m * P : (m + 1) * P],
                                        xgtsb[k][:, j, boff : boff + tb],
                                        start=(k == 0 and j == 0),
                                        stop=(k == HK2 - 1 and j == 1),
                                    )
                        nc.scalar.activation(
                            interT[:, m, boff : boff + tb],
                            ps[:, :tb],
                            act,
                            bias=b1sb[:, m : m + 1],
                            scale=1.0 / W1S,
                        )

                pending = None
                final_mm1_hoisted = len(blocks) > 1
                for bi, (boff, tb) in enumerate(blocks):
                    nch = (tb + P - 1) // P
                    if not (final_mm1_hoisted and bi == len(blocks) - 1):
                        emit_mm1(boff, tb)

                    if pending is not None:
                        emit_normalize(*pending)
                        pending = None

                    s1 = spool.tile([P, 4], F32, tag="s1")
                    s2 = spool.tile([P, 4], F32, tag="s2")
                    xs = []
                    for tci in range(nch):
                        tcg = boff // P + tci
                        toff = tci * P
                        tw = min(P, tb - toff)
                        if bi == len(blocks) - 2 and tci == nch - 1:
                            # final (small) block's matmul1+gelu slot in just
                            # before this block's last matmul2 chunk, so its
                            # gelus overlap that chunk's PE time instead of
                            # serializing after it.
                            emit_mm1(*blocks[-1])
                        psy = ppb.tile([P, H], F32, tag="psB")
                        if MM2_K_OUTER:
                            for k in range(IK):
                                for n0, nw in ((0, 512), (512, 256)):
                                    nc.tensor.matmul(
                                        psy[:tw, n0 : n0 + nw],
                                        interT[:, k, boff + toff : boff + toff + tw],
                                        w2sb[k][:, n0 : n0 + nw],
                                        start=(k == 0),
                                        stop=(k == IK - 1),
                                        skip_group_check=True,
                                    )
                        else:
                            for n0, nw in ((0, 512), (512, 256)):
                                for k in range(IK):
                                    nc.tensor.matmul(
                                        psy[:tw, n0 : n0 + nw],
                                        interT[:, k, boff + toff : boff + toff + tw],
                                        w2sb[k][:, n0 : n0 + nw],
                                        start=(k == 0),
                                        stop=(k == IK - 1),
                                    )
                        x = lnpool.tile([P, H], F32, tag=f"x{tci}")
                        xs.append(x)
                        # residual add with fused row-sum: one DVE pass gives
                        # both x = psy + xres and s1 = sum_h(x)
                        nc.vector.scalar_tensor_tensor(
                            x[:tw],
                            psy[:tw],
                            0.0,
                            xressb[tcg][:tw],
                            op0=ALU.bypass,
                            op1=ALU.add,
                            accum_out=s1[:tw, tci : tci + 1],
                        )
                        sq = lnpool.tile([P, H], BF16, tag="sq")
                        nc.scalar.activation(
                            sq[:tw], x[:tw], AF.Square, accum_out=s2[:tw, tci : tci + 1]
                        )

                    # block stats: mu_neg = -s1/H, var+eps = s2/H + eps - mu^2,
                    # rs = rsqrt(var+eps) via bit trick + 2 Newton iterations.
                    mun = spool.tile([P, 4], F32, tag="mun")
                    nc.vector.tensor_scalar_mul(mun[:, :nch], s1[:, :nch], -1.0 / H)
                    ex2 = spool.tile([P, 4], F32, tag="ex2")
                    nc.vector.tensor_scalar(
                        ex2[:, :nch], s2[:, :nch], 1.0 / H, EPS, op0=ALU.mult, op1=ALU.add
                    )
                    ve = spool.tile([P, 4], F32, tag="ve")
                    nc.vector.tensor_mul(ve[:, :nch], mun[:, :nch], mun[:, :nch])
                    nc.vector.tensor_sub(ve[:, :nch], ex2[:, :nch], ve[:, :nch])
                    yt = spool.tile([P, 4], F32, tag="yt")
                    nc.vector.tensor_tensor(
                        yt[:, :nch].bitcast(I32),
                        ve[:, :nch].bitcast(I32),
                        onesi[:, :nch],
                        op=ALU.logical_shift_right,
                    )
                    nc.vector.tensor_sub(
                        yt[:, :nch].bitcast(I32),
                        magici[:, :nch],
                        yt[:, :nch].bitcast(I32),
                    )
                    vh = spool.tile([P, 4], F32, tag="vh")
                    nc.vector.tensor_scalar_mul(vh[:, :nch], ve[:, :nch], 0.5)
                    tta = spool.tile([P, 4], F32, tag="tta")
                    if nch == 1:
                        # single-chunk (tail) block: yt is [P,1], so it can ride
                        # the STT scalar slot — 2 fused ops per Newton step.
                        # Step A: t = (y*y)*vh.  Step B: y' = (t - 1.5)*y = -y_next;
                        # the sign cancels inside step A of the next iteration and
                        # an even iteration count ends positive.
                        for _newton in range(2):
                            nc.vector.scalar_tensor_tensor(
                                tta[:, :1], yt[:, :1], yt[:, :1], vh[:, :1],
                                op0=ALU.mult, op1=ALU.mult,
                            )
                            nc.vector.scalar_tensor_tensor(
                                yt[:, :1], tta[:, :1], 1.5, yt[:, :1],
                                op0=ALU.subtract, op1=ALU.mult,
                            )
                    else:
                        for _newton in range(2):
                            nc.vector.tensor_mul(tta[:, :nch], yt[:, :nch], yt[:, :nch])
                            nc.vector.tensor_mul(tta[:, :nch], tta[:, :nch], vh[:, :nch])
                            nc.vector.tensor_sub(tta[:, :nch], c15[:, :nch], tta[:, :nch])
                            nc.vector.tensor_mul(yt[:, :nch], yt[:, :nch], tta[:, :nch])
                    nmr = spool.tile([P, 4], F32, tag="nmr")
                    nc.vector.tensor_mul(nmr[:, :nch], mun[:, :nch], yt[:, :nch])

                    pending = (boff, tb, nch, xs, yt, nmr)
                if pending is not None:
                    emit_normalize(*pending)

    nc.finalize()
    return nc


_NC_CACHE: dict[tuple, object] = {}


def _get_nc(C: int, n_tok: int, reps: int = 1, affine: bool = True):
    key = (C, n_tok, reps, affine)
    if key not in _NC_CACHE:
        _NC_CACHE[key] = _build(C, reps=reps, n_tok=n_tok, affine=affine)
    return _NC_CACHE[key]


def _prepare(hidden_states, mask, w1, b1, w2, b2, ln_gamma, ln_beta, reps=1):
    hs = np.asarray(hidden_states, dtype=np.float32)
    mk = np.asarray(mask).reshape(-1).astype(np.int64)
    w1 = np.asarray(w1, dtype=np.float32)
    b1 = np.asarray(b1, dtype=np.float32)
    w2 = np.asarray(w2, dtype=np.float32)
    b2 = np.asarray(b2, dtype=np.float32)
    g = np.asarray(ln_gamma, dtype=np.float32)
    bt = np.asarray(ln_beta, dtype=np.float32)

    idxs = [np.nonzero(mk == e)[0] for e in range(E)]
    max_n = max(len(ix) for ix in idxs)
    C = max(256, -(-max_n // P) * P)  # DRAM capacity: multiple of 128
    n_tok = max(256, max_n)  # tokens actually computed
    # identity-affine fast path: gamma==1, beta==0 makes the post-LN affine a
    # no-op, so the specialized module skips those two DVE passes per chunk
    affine = not (np.all(g == 1.0) and np.all(bt == 0.0))
    nc = _get_nc(C, n_tok, reps, affine)
    TCN = C // P

    gb = np.empty((P, 2, H), dtype=np.float32)
    gb[:, 0, :] = g[None, :]
    gb[:, 1, :] = bt[None, :]

    hs2 = hs.reshape(T, H)
    in_maps = []
    for e in range(E):
        ix = idxs[e]
        xg = np.zeros((C, H), dtype=np.float32)
        xg[: len(ix)] = hs2[ix]
        # feature-major x, packed for DoubleRow: [k2, p, j, t] <- h = k2*256 + j*128 + p
        xgt = np.ascontiguousarray(
            xg.T.reshape(HK2, 2, P, C).transpose(0, 2, 1, 3)
        ).astype(ml_dtypes.float8_e4m3)
        w1p = np.ascontiguousarray(
            (w1[e] * W1S).reshape(HK2, 2, P, I).transpose(0, 2, 1, 3)
        ).astype(ml_dtypes.float8_e4m3)
        xres = (xg + b2[e][None, :]).astype(ml_dtypes.bfloat16).reshape(TCN, P, H)
        im = {
            "xgt": xgt,
            "xres": xres,
            "w1": w1p,
            "b1t": np.ascontiguousarray(b1[e].reshape(IK, P).T),
            "w2": w2[e].astype(ml_dtypes.bfloat16).reshape(IK, P, H),
        }
        if affine:
            im["gb"] = gb
        in_maps.append(im)

    return nc, in_maps, idxs, C


def _scatter(res, idxs, C):
    out = np.empty((T, H), dtype=np.float32)
    for e in range(E):
        ix = idxs[e]
        out[ix] = res.results[e]["out"].reshape(C, H)[: len(ix)]
    return out.reshape(1, T, H)


def kernel(**inputs):
    nc, in_maps, idxs, C = _prepare(**inputs)
    res = run_bass_kernel_spmd(nc, in_maps, list(range(N_CORES)))
    return _scatter(res, idxs, C)
